# revision 6
# baseline (speedup 1.0000x reference)
"""CrossSetNorm Trainium2 kernel v4 (8 NeuronCores, batch-parallel).

Problem: x [2048, 328, 256] f32, mask [2048, 328] bool (True = dead).
Two independent masked set-norms over the set dim per sample:
  obj = s in [0, 128), road = s in [128, 328)
out[s,d] = xm[s,d] * A[samp,d] + C[samp,d] with A = istd*w, C = b - mean*A.

v4 design (per core, 256 samples = 8 chunks of 32):
  - HOST: pre-mask xm = x*alive, cast bf16, layout [chunk, D, pair, S, par]
    so d is on partitions and each pair of samples is interleaved along
    the innermost axis. bn_stats' even/odd split then yields BOTH
    samples' (count, mean, M2) in ONE op -> 512 bn ops instead of 1024.
  - counts: alive shipped transposed [S, B]; per-sample alive counts via
    3 PE ones-matmuls (contract over s on partitions) into PSUM [1,2,256].
    Count-derived rows (nfull*rc, g, rc, okt, okm) computed once on DVE
    partition-1 tiles, cast to f32r, and PE-broadcast to 128 partitions
    per chunk (ones [1,128] matmul) -- replaces v3's 160 SEL matmuls.
  - chain per chunk on [128, 2h, 16p, 2seg, 2par] f32 views (h-batched).
  - apply: per (sample, seg, h) fused mult-add split across DVE
    tensor_scalar / ScalarE activation / GpSimd tensor_scalar by a
    scrambled modular schedule (N_DVE/N_SC of 64 slots per chunk-half).
  - DMA: 2 loads + 2 stores per chunk of [128, 21KB] (128 descriptors
    of 20992B) -- SP HWDGE in, gpsimd SWDGE out.
"""
import sys

if "/opt/trn_rl_repo" not in sys.path:
    sys.path.insert(0, "/opt/trn_rl_repo")

from contextlib import ExitStack

import numpy as np
import ml_dtypes

import concourse.bacc as bacc
import concourse.bass as bass
import concourse.tile as tile
from concourse import mybir
from concourse.bass_utils import run_bass_kernel_spmd

F32 = mybir.dt.float32
F32R = mybir.dt.float32r
BF16 = mybir.dt.bfloat16
AF = mybir.ActivationFunctionType
OP = mybir.AluOpType
AX = mybir.AxisListType
BF = ml_dtypes.bfloat16

NCORES = 8
B, S, D = 2048, 328, 256
B_LOC = B // NCORES      # 256
S_OBJ = 128
S_ROAD = S - S_OBJ       # 200
CHUNK = 32               # samples per chunk
NPAIR = CHUNK // 2       # 16 sample pairs per chunk
NCHUNK = B_LOC // CHUNK  # 8
EPS = 1e-6

# apply engine split: of each 64 (pair, par, seg) slots per (chunk, h):
# first N_DVE -> DVE, next N_SC -> ScalarE, rest -> GpSimd
N_DVE = 2
N_SC = 39

_NC_CACHE = {}


def _owner(c, h, p, par, seg):
    r = (4 * p + 2 * par + seg + 7 * h + 13 * c) % 64
    if r < N_DVE:
        return "v"
    if r < N_DVE + N_SC:
        return "s"
    return "g"


def build_nc():
    nc = bacc.Bacc("TRN2", target_bir_lowering=False, debug=False, num_devices=NCORES)
    x_d = nc.declare_dram_parameter("x", [NCHUNK, D, NPAIR, S, 2], BF16, isOutput=False)
    at_d = nc.declare_dram_parameter("alive_t", [S, B_LOC], BF16, isOutput=False)
    w_obj_d = nc.declare_dram_parameter("weights_obj", [D], F32, isOutput=False)
    b_obj_d = nc.declare_dram_parameter("biases_obj", [D], F32, isOutput=False)
    w_road_d = nc.declare_dram_parameter("weights_road", [D], F32, isOutput=False)
    b_road_d = nc.declare_dram_parameter("biases_road", [D], F32, isOutput=False)
    out_d = nc.declare_dram_parameter("out", [NCHUNK, D, NPAIR, S, 2], BF16, isOutput=True)

    with tile.TileContext(nc) as tc, ExitStack() as ctx:
        singles = ctx.enter_context(tc.tile_pool(name="singles", bufs=1))
        xp = ctx.enter_context(tc.tile_pool(name="xp", bufs=6))
        bnp = ctx.enter_context(tc.tile_pool(name="bnp", bufs=2))
        tmp = ctx.enter_context(tc.tile_pool(name="tmp", bufs=2))
        psA = ctx.enter_context(tc.tile_pool(name="psA", bufs=1, space="PSUM"))
        psB = ctx.enter_context(tc.tile_pool(name="psB", bufs=2, space="PSUM"))

        # ---- constants ----
        onesK = singles.tile([128, 1], BF16)
        nc.vector.memset(onesK, 1.0)
        onesM = singles.tile([1, 128], F32)
        nc.vector.memset(onesM, 1.0)
        nfullT = singles.tile([1, 2, B_LOC], F32)
        nc.vector.memset(nfullT[:, 0:1, :], float(S_OBJ))
        nc.vector.memset(nfullT[:, 1:2, :], float(S_ROAD))
        W = singles.tile([128, 2, 2], F32)   # [d-half, h, seg]
        Bt = singles.tile([128, 2, 2], F32)
        for h in range(2):
            for seg, (wd, bd) in enumerate(((w_obj_d, b_obj_d), (w_road_d, b_road_d))):
                nc.sync.dma_start(
                    out=W[:, h : h + 1, seg : seg + 1],
                    in_=bass.AP(tensor=wd, offset=128 * h, ap=[[1, 128], [1, 1]]),
                )
                nc.sync.dma_start(
                    out=Bt[:, h : h + 1, seg : seg + 1],
                    in_=bass.AP(tensor=bd, offset=128 * h, ap=[[1, 128], [1, 1]]),
                )
        aT1 = singles.tile([128, 2, B_LOC], BF16)
        nc.sync.dma_start(out=aT1[:, 0:1, :], in_=at_d[0:128, :])
        nc.sync.dma_start(out=aT1[:, 1:2, :], in_=at_d[128:256, :])
        aT2 = singles.tile([128, B_LOC], BF16)
        nc.sync.dma_start(out=aT2[0:72, :], in_=at_d[256:328, :])

        # ---- x loads ----
        x_tiles = {}

        def emit_loads(c):
            for h in range(2):
                t = xp.tile([128, NPAIR, S, 2], BF16, tag="x", name="x")
                nc.sync.dma_start(out=t, in_=x_d[c, 128 * h : 128 * (h + 1), :, :, :])
                x_tiles[(c, h)] = t

        emit_loads(0)

        # ---- alive counts via PE (contract over s on partitions) ----
        CNT = psA.tile([1, 2, B_LOC], F32, tag="cnt")
        nc.tensor.matmul(CNT[:, 0:1, :], onesK[:, :], aT1[:, 0:1, :], start=True, stop=True)
        nc.tensor.matmul(CNT[:, 1:2, :], onesK[:, :], aT1[:, 1:2, :], start=True, stop=False)
        nc.tensor.matmul(CNT[:, 1:2, :], onesK[0:72, :], aT2[0:72, :], start=False, stop=True)

        # ---- count-derived rows, once per core on [1, 2, 256] ----
        # rows: 0 = nfull*rc (mean scale), 1 = nfull*rc - 2, 2 = rc,
        #       3 = okt (1 if count > 1 else 0), 4 = okm = 1 - okt
        RF = singles.tile([1, 5, 2, B_LOC], F32)
        cc = singles.tile([1, 2, B_LOC], F32)
        nc.vector.tensor_scalar(cc, CNT, 1.0, None, OP.max)
        nc.vector.reciprocal(RF[:, 2:3, :, :], cc)
        nc.vector.tensor_mul(RF[:, 0:1, :, :], RF[:, 2:3, :, :], nfullT)
        nc.vector.tensor_scalar(RF[:, 1:2, :, :], RF[:, 0:1, :, :], -2.0, None, OP.add)
        nc.vector.tensor_scalar(RF[:, 3:4, :, :], CNT, -1.0, 1.0, OP.add, OP.min)
        nc.vector.tensor_scalar(RF[:, 3:4, :, :], RF[:, 3:4, :, :], 0.0, None, OP.max)
        nc.vector.tensor_scalar(RF[:, 4:5, :, :], RF[:, 3:4, :, :], -1.0, 1.0, OP.mult, OP.add)
        R = singles.tile([1, 5, 2, NCHUNK, NPAIR, 2], F32R)
        nc.vector.tensor_scalar(R[:, :, :, :, :, :], RF, 1.0, None, OP.mult)

        for c in range(NCHUNK):
            if c + 1 < NCHUNK:
                emit_loads(c + 1)

            # ---- broadcast count rows to 128 partitions for this chunk ----
            RB = psB.tile([128, 5, 2, NPAIR, 2], F32, tag="rb", name="RB")
            nc.tensor.matmul(
                RB, onesM[:, :].bitcast(F32R), R[0:1, :, :, c, :, :],
                start=True, stop=True,
            )
            Rsb = tmp.tile([128, 2, 5, NPAIR, 2, 2], F32, name="rsb")
            for h in range(2):
                for seg in range(2):
                    nc.vector.tensor_scalar(
                        Rsb[:, h : h + 1, :, :, seg : seg + 1, :],
                        RB[:, :, seg : seg + 1, :, :],
                        1.0, None, OP.mult,
                    )

            def row(r):
                return Rsb[:, :, r : r + 1, :, :, :]

            # ---- bn_stats: one op per (h, pair, seg); even/odd = par ----
            BN = bnp.tile([128, 2, NPAIR, 2, 2, 3], F32, tag="bn", name="BN")
            for h in range(2):
                xg = x_tiles[(c, h)]
                for p in range(NPAIR):
                    nc.vector.bn_stats(
                        BN[:, h : h + 1, p : p + 1, 0:1, :, :].opt(),
                        xg[:, p : p + 1, 0:S_OBJ, :].opt(),
                    )
                    nc.vector.bn_stats(
                        BN[:, h : h + 1, p : p + 1, 1:2, :, :].opt(),
                        xg[:, p : p + 1, S_OBJ:S, :].opt(),
                    )

            # ---- stats chain, h-batched on [128, 2, 16, 2, 2] f32 ----
            me = BN[:, :, :, :, :, 1:2]
            Me = BN[:, :, :, :, :, 2:3]
            MEAN = tmp.tile([128, 2, NPAIR, 2, 2], F32, name="MEAN")
            A1 = tmp.tile([128, 2, NPAIR, 2, 2], F32, name="A1")
            B1 = tmp.tile([128, 2, NPAIR, 2, 2], F32, name="B1")
            U = tmp.tile([128, 2, NPAIR, 2, 2], F32, name="U")
            ISTD = tmp.tile([128, 2, NPAIR, 2, 2], F32, name="ISTD")
            At = tmp.tile([128, 2, NPAIR, 2, 2], F32, name="At")
            Ct = tmp.tile([128, 2, NPAIR, 2, 2], F32, name="Ct")
            nc.vector.tensor_mul(MEAN, me, row(0))       # mean
            nc.vector.tensor_mul(A1, me, me)
            nc.vector.tensor_mul(A1, A1, row(0))         # me^2*nfull*rc
            nc.vector.tensor_mul(U, Me, row(2))          # M2*rc
            nc.vector.tensor_add(U, U, A1)               # s2*rc
            nc.vector.tensor_mul(B1, MEAN, MEAN)
            nc.vector.tensor_mul(B1, B1, row(1))         # mean^2*(n*rc-2)
            nc.vector.tensor_add(U, U, B1)               # var
            nc.vector.tensor_scalar(U, U, EPS, None, OP.add)
            nc.vector.reciprocal(B1, U)                  # 1/(var+eps)
            nc.scalar.activation(ISTD, B1, AF.Sqrt)      # istd
            nc.vector.tensor_mul(ISTD, ISTD, row(3))     # *okt
            nc.vector.tensor_add(ISTD, ISTD, row(4))     # +okm
            for h in range(2):
                for seg in range(2):
                    nc.vector.tensor_scalar(
                        At[:, h : h + 1, :, seg : seg + 1, :],
                        ISTD[:, h : h + 1, :, seg : seg + 1, :],
                        W[:, h : h + 1, seg : seg + 1], None, OP.mult,
                    )
            nc.vector.tensor_mul(Ct, MEAN, At)
            for h in range(2):
                for seg in range(2):
                    nc.vector.tensor_scalar(
                        Ct[:, h : h + 1, :, seg : seg + 1, :],
                        Ct[:, h : h + 1, :, seg : seg + 1, :],
                        -1.0, Bt[:, h : h + 1, seg : seg + 1], OP.mult, OP.add,
                    )

            # ---- apply + store ----
            for h in range(2):
                xg = x_tiles.pop((c, h))
                for p in range(NPAIR):
                    for par in range(2):
                        for seg in range(2):
                            if seg == 0:
                                sl = xg[:, p : p + 1, 0:S_OBJ, par : par + 1]
                            else:
                                sl = xg[:, p : p + 1, S_OBJ:S, par : par + 1]
                            a = At[:, h : h + 1, p : p + 1, seg : seg + 1, par : par + 1]
                            cv = Ct[:, h : h + 1, p : p + 1, seg : seg + 1, par : par + 1]
                            o = _owner(c, h, p, par, seg)
                            if o == "s":
                                nc.scalar.activation(
                                    sl, sl, AF.Identity, bias=cv, scale=a
                                )
                            elif o == "v":
                                nc.vector.tensor_scalar(sl, sl, a, cv, OP.mult, OP.add)
                            else:
                                nc.gpsimd.tensor_scalar(sl, sl, a, cv, OP.mult, OP.add)
                nc.gpsimd.dma_start(
                    out=out_d[c, 128 * h : 128 * (h + 1), :, :, :], in_=xg[:, :, :, :]
                )

    nc.compile()
    return nc


def _get_nc():
    if "nc" not in _NC_CACHE:
        _NC_CACHE["nc"] = build_nc()
    return _NC_CACHE["nc"]


def kernel(x, mask, weights_obj, biases_obj, weights_road, biases_road, _trace=False):
    x = np.asarray(x, dtype=np.float32)
    mask = np.asarray(mask).astype(bool)
    w_obj = np.ascontiguousarray(np.asarray(weights_obj, dtype=np.float32))
    b_obj = np.ascontiguousarray(np.asarray(biases_obj, dtype=np.float32))
    w_road = np.ascontiguousarray(np.asarray(weights_road, dtype=np.float32))
    b_road = np.ascontiguousarray(np.asarray(biases_road, dtype=np.float32))

    xm = np.where(mask[:, :, None], np.float32(0), x).astype(BF)
    # [B, S, D] -> [core, chunk, pair, par, S, D] -> [core, chunk, D, pair, S, par]
    xt = np.ascontiguousarray(
        xm.reshape(NCORES, NCHUNK, NPAIR, 2, S, D).transpose(0, 1, 5, 2, 4, 3)
    )
    alive_t = np.ascontiguousarray(
        (~mask).astype(BF).reshape(NCORES, B_LOC, S).transpose(0, 2, 1)
    )

    in_maps = [
        {
            "x": xt[i],
            "alive_t": alive_t[i],
            "weights_obj": w_obj,
            "biases_obj": b_obj,
            "weights_road": w_road,
            "biases_road": b_road,
        }
        for i in range(NCORES)
    ]
    nc = _get_nc()
    res = run_bass_kernel_spmd(nc, in_maps, core_ids=list(range(NCORES)), trace=_trace)
    outs = []
    for i in range(NCORES):
        o = np.asarray(res.results[i]["out"])  # [NCHUNK, D, NPAIR, S, 2] bf16
        outs.append(
            o.transpose(0, 2, 4, 3, 1).reshape(B_LOC, S, D)
        )
    out = np.concatenate(outs, axis=0).astype(np.float32)
    if _trace:
        kernel.last_exec_time_ns = res.exec_time_ns
        kernel.last_mean_exec_time_ns = res.mean_exec_time_ns
    return out.reshape(B, S, D)


# revision 11
# speedup vs baseline: 1.0607x; 1.0607x over previous
"""CrossSetNorm Trainium2 kernel v6 (8 NeuronCores, batch-parallel).

Problem: x [2048, 328, 256] f32, mask [2048, 328] bool (True = dead).
Two independent masked set-norms over the set dim per sample:
  obj = s in [0, 128), road = s in [128, 328)
out[s,d] = xm[s,d] * A[samp,d] + C[samp,d] with A = istd*w, C = b - mean*A.

v6 design (per core, 256 samples = 8 chunks of 32):
  - HOST: pre-mask xm = x*alive, cast bf16, layout [chunk, D, samp, S]
    (d on partitions, contiguous per-sample rows -> full-rate apply).
  - bn_stats per (sample, seg, h): even/odd 6-tuple recombined in the
    chain (v3 math). 1024 ops on DVE -- the dominant cost; everything
    else is kept off DVE where possible.
  - counts: alive shipped transposed [S, B]; per-sample alive counts via
    3 PE ones-matmuls into PSUM [1,2,256]; count-derived rows computed
    once on DVE and PE-broadcast to 128 partitions per chunk (replaces
    v3's 160 SEL matmuls + pack/transpose machinery).
  - chain h-batched per chunk on [128, 2, 32, 2] f32 views; chunk 0 is
    processed in two 16-sample half-ranges so ScalarE/GpSimd applies
    start ~25us earlier (pipeline warmup).
  - apply: per (sample, seg, h) fused mult-add split across DVE/SC/GP by
    per-chunk tables (DVE-heavy in the last chunks to kill the tail).
  - DMA: 2 loads + 2 stores per chunk of [128, 21KB] (128 descriptors of
    20992B each); SP HWDGE in, gpsimd SWDGE out.
"""
import sys

if "/opt/trn_rl_repo" not in sys.path:
    sys.path.insert(0, "/opt/trn_rl_repo")

from contextlib import ExitStack

import numpy as np
import ml_dtypes

import concourse.bacc as bacc
import concourse.bass as bass
import concourse.tile as tile
from concourse import mybir
from concourse.bass_utils import run_bass_kernel_spmd

F32 = mybir.dt.float32
F32R = mybir.dt.float32r
BF16 = mybir.dt.bfloat16
AF = mybir.ActivationFunctionType
OP = mybir.AluOpType
BF = ml_dtypes.bfloat16

NCORES = 8
B, S, D = 2048, 328, 256
B_LOC = B // NCORES      # 256
S_OBJ = 128
S_ROAD = S - S_OBJ       # 200
CHUNK = 32               # samples per chunk
NCHUNK = B_LOC // CHUNK  # 8
EPS = 1e-6

# apply split per chunk: of each 64 (j, seg) slots per (chunk, h):
# first N_DVE -> DVE, next N_SC -> ScalarE, rest -> GpSimd
N_DVE_TAB = [0, 2, 2, 2, 2, 2, 10, 22]
N_SC_TAB = [40, 39, 39, 39, 39, 39, 36, 30]

_NC_CACHE = {}


def _owner(c, h, j, seg):
    r = (2 * j + seg + 7 * h + 13 * c) % 64
    nd, ns = N_DVE_TAB[c], N_SC_TAB[c]
    if r < nd:
        return "v"
    if r < nd + ns:
        return "s"
    return "g"


def build_nc():
    nc = bacc.Bacc("TRN2", target_bir_lowering=False, debug=False, num_devices=NCORES)
    x_d = nc.declare_dram_parameter("x", [NCHUNK, D, CHUNK, S], BF16, isOutput=False)
    at_d = nc.declare_dram_parameter("alive_t", [S, B_LOC], BF16, isOutput=False)
    w_obj_d = nc.declare_dram_parameter("weights_obj", [D], F32, isOutput=False)
    b_obj_d = nc.declare_dram_parameter("biases_obj", [D], F32, isOutput=False)
    w_road_d = nc.declare_dram_parameter("weights_road", [D], F32, isOutput=False)
    b_road_d = nc.declare_dram_parameter("biases_road", [D], F32, isOutput=False)
    out_d = nc.declare_dram_parameter("out", [NCHUNK, D, CHUNK, S], BF16, isOutput=True)

    with tile.TileContext(nc) as tc, ExitStack() as ctx:
        singles = ctx.enter_context(tc.tile_pool(name="singles", bufs=1))
        xp = ctx.enter_context(tc.tile_pool(name="xp", bufs=6))
        bnp = ctx.enter_context(tc.tile_pool(name="bnp", bufs=2))
        tmp = ctx.enter_context(tc.tile_pool(name="tmp", bufs=2))
        psA = ctx.enter_context(tc.tile_pool(name="psA", bufs=1, space="PSUM"))
        psB = ctx.enter_context(tc.tile_pool(name="psB", bufs=2, space="PSUM"))

        # ---- constants ----
        onesK = singles.tile([128, 1], BF16)
        nc.vector.memset(onesK, 1.0)
        onesM = singles.tile([1, 128], F32)
        nc.vector.memset(onesM, 1.0)
        nhalfT = singles.tile([1, 2, B_LOC], F32)
        nc.vector.memset(nhalfT[:, 0:1, :], float(S_OBJ // 2))
        nc.vector.memset(nhalfT[:, 1:2, :], float(S_ROAD // 2))
        nfullT = singles.tile([1, 2, B_LOC], F32)
        nc.vector.memset(nfullT[:, 0:1, :], float(S_OBJ))
        nc.vector.memset(nfullT[:, 1:2, :], float(S_ROAD))
        W = singles.tile([128, 2, 2], F32)   # [d-half, h, seg]
        Bt = singles.tile([128, 2, 2], F32)
        for h in range(2):
            for seg, (wd, bd) in enumerate(((w_obj_d, b_obj_d), (w_road_d, b_road_d))):
                nc.sync.dma_start(
                    out=W[:, h : h + 1, seg : seg + 1],
                    in_=bass.AP(tensor=wd, offset=128 * h, ap=[[1, 128], [1, 1]]),
                )
                nc.sync.dma_start(
                    out=Bt[:, h : h + 1, seg : seg + 1],
                    in_=bass.AP(tensor=bd, offset=128 * h, ap=[[1, 128], [1, 1]]),
                )
        aT1 = singles.tile([128, 2, B_LOC], BF16)
        nc.sync.dma_start(out=aT1[:, 0:1, :], in_=at_d[0:128, :])
        nc.sync.dma_start(out=aT1[:, 1:2, :], in_=at_d[128:256, :])
        aT2 = singles.tile([128, B_LOC], BF16)
        nc.sync.dma_start(out=aT2[0:72, :], in_=at_d[256:328, :])

        # ---- x loads ----
        x_tiles = {}

        def emit_loads(c):
            for h in range(2):
                t = xp.tile([128, CHUNK, S], BF16, tag="x", name="x")
                nc.sync.dma_start(out=t, in_=x_d[c, 128 * h : 128 * (h + 1), :, :])
                x_tiles[(c, h)] = t

        emit_loads(0)

        # ---- alive counts via PE (contract over s on partitions) ----
        CNT = psA.tile([1, 2, B_LOC], F32, tag="cnt")
        nc.tensor.matmul(CNT[:, 0:1, :], onesK[:, :], aT1[:, 0:1, :], start=True, stop=True)
        nc.tensor.matmul(CNT[:, 1:2, :], onesK[:, :], aT1[:, 1:2, :], start=True, stop=False)
        nc.tensor.matmul(CNT[:, 1:2, :], onesK[0:72, :], aT2[0:72, :], start=False, stop=True)

        # ---- count-derived rows, once per core on [1, 2, 256] ----
        # rows: 0 = nhalf*rc, 1 = nfull*rc - 2, 2 = rc, 3 = okt, 4 = okm
        RF = singles.tile([1, 5, 2, B_LOC], F32)
        cc = singles.tile([1, 2, B_LOC], F32)
        nc.vector.tensor_scalar(cc, CNT, 1.0, None, OP.max)
        nc.vector.reciprocal(RF[:, 2:3, :, :], cc)
        nc.vector.tensor_mul(RF[:, 0:1, :, :], RF[:, 2:3, :, :], nhalfT)
        nc.vector.tensor_mul(RF[:, 1:2, :, :], RF[:, 2:3, :, :], nfullT)
        nc.vector.tensor_scalar(RF[:, 1:2, :, :], RF[:, 1:2, :, :], -2.0, None, OP.add)
        nc.vector.tensor_scalar(RF[:, 3:4, :, :], CNT, -1.0, 1.0, OP.add, OP.min)
        nc.vector.tensor_scalar(RF[:, 3:4, :, :], RF[:, 3:4, :, :], 0.0, None, OP.max)
        nc.vector.tensor_scalar(RF[:, 4:5, :, :], RF[:, 3:4, :, :], -1.0, 1.0, OP.mult, OP.add)
        R = singles.tile([1, 5, 2, NCHUNK, CHUNK], F32R)
        nc.vector.tensor_scalar(R[:, :, :, :, :], RF, 1.0, None, OP.mult)

        for c in range(NCHUNK):
            if c + 1 < NCHUNK:
                emit_loads(c + 1)

            # ---- broadcast count rows to 128 partitions for this chunk ----
            RB = psB.tile([128, 5, 2, CHUNK], F32, tag="rb", name="RB")
            nc.tensor.matmul(
                RB, onesM[:, :].bitcast(F32R), R[0:1, :, :, c, :],
                start=True, stop=True,
            )
            # Rsb: [128, h, 5, samp, seg] (rows duplicated per d-half)
            Rsb = tmp.tile([128, 2, 5, CHUNK, 2], F32, name="rsb")
            for h in range(2):
                for seg in range(2):
                    nc.vector.tensor_scalar(
                        Rsb[:, h : h + 1, :, :, seg : seg + 1],
                        RB[:, :, seg : seg + 1, :],
                        1.0, None, OP.mult,
                    )

            BN = bnp.tile([128, 2, CHUNK, 2, 6], F32, tag="bn", name="BN")
            MEAN = tmp.tile([128, 2, CHUNK, 2], F32, name="MEAN")
            A1 = tmp.tile([128, 2, CHUNK, 2], F32, name="A1")
            B1 = tmp.tile([128, 2, CHUNK, 2], F32, name="B1")
            U = tmp.tile([128, 2, CHUNK, 2], F32, name="U")
            ISTD = tmp.tile([128, 2, CHUNK, 2], F32, name="ISTD")
            At = tmp.tile([128, 2, CHUNK, 2], F32, name="At")
            Ct = tmp.tile([128, 2, CHUNK, 2], F32, name="Ct")

            # chunk 0 processed in two half-ranges for faster pipeline start
            ranges = [(0, 16), (16, 32)] if c == 0 else [(0, CHUNK)]
            for (s0, s1) in ranges:
                # ---- bn_stats per (h, sample, seg) ----
                for h in range(2):
                    xg = x_tiles[(c, h)]
                    for j in range(s0, s1):
                        nc.vector.bn_stats(
                            BN[:, h : h + 1, j : j + 1, 0:1, :],
                            xg[:, j : j + 1, 0:S_OBJ],
                        )
                        nc.vector.bn_stats(
                            BN[:, h : h + 1, j : j + 1, 1:2, :],
                            xg[:, j : j + 1, S_OBJ:S],
                        )

                # ---- stats chain, h-batched on [128, 2, samp, 2] views ----
                def rr(r):
                    return Rsb[:, :, r : r + 1, s0:s1, :]

                sl_ = (slice(None), slice(None), slice(s0, s1), slice(None))
                me = BN[:, :, s0:s1, :, 1:2]
                mo = BN[:, :, s0:s1, :, 4:5]
                Me = BN[:, :, s0:s1, :, 2:3]
                Mo = BN[:, :, s0:s1, :, 5:6]
                nc.vector.tensor_add(U[sl_], me, mo)
                nc.vector.tensor_mul(MEAN[sl_], U[sl_], rr(0))   # mean
                nc.vector.tensor_mul(A1[sl_], me, me)
                nc.vector.tensor_mul(B1[sl_], mo, mo)
                nc.vector.tensor_add(A1[sl_], A1[sl_], B1[sl_])
                nc.vector.tensor_mul(A1[sl_], A1[sl_], rr(0))    # (me^2+mo^2)*nh*rc
                nc.vector.tensor_add(B1[sl_], Me, Mo)
                nc.vector.tensor_mul(B1[sl_], B1[sl_], rr(2))    # M2*rc
                nc.vector.tensor_add(A1[sl_], A1[sl_], B1[sl_])  # s2*rc
                nc.vector.tensor_mul(B1[sl_], MEAN[sl_], MEAN[sl_])
                nc.vector.tensor_mul(B1[sl_], B1[sl_], rr(1))    # mean^2*(n*rc-2)
                nc.vector.tensor_add(A1[sl_], A1[sl_], B1[sl_])  # var
                nc.vector.tensor_scalar(A1[sl_], A1[sl_], EPS, None, OP.add)
                nc.vector.reciprocal(B1[sl_], A1[sl_])
                nc.scalar.activation(ISTD[sl_], B1[sl_], AF.Sqrt)
                nc.vector.tensor_mul(ISTD[sl_], ISTD[sl_], rr(3))
                nc.vector.tensor_add(ISTD[sl_], ISTD[sl_], rr(4))
                for h in range(2):
                    for seg in range(2):
                        nc.vector.tensor_scalar(
                            At[:, h : h + 1, s0:s1, seg : seg + 1],
                            ISTD[:, h : h + 1, s0:s1, seg : seg + 1],
                            W[:, h : h + 1, seg : seg + 1], None, OP.mult,
                        )
                nc.vector.tensor_mul(B1[sl_], MEAN[sl_], At[sl_])
                for h in range(2):
                    for seg in range(2):
                        nc.vector.tensor_scalar(
                            Ct[:, h : h + 1, s0:s1, seg : seg + 1],
                            B1[:, h : h + 1, s0:s1, seg : seg + 1],
                            -1.0, Bt[:, h : h + 1, seg : seg + 1], OP.mult, OP.add,
                        )

                # ---- apply ----
                for h in range(2):
                    xg = x_tiles[(c, h)]
                    for j in range(s0, s1):
                        for seg in range(2):
                            if seg == 0:
                                sl = xg[:, j : j + 1, 0:S_OBJ]
                            else:
                                sl = xg[:, j : j + 1, S_OBJ:S]
                            a = At[:, h : h + 1, j : j + 1, seg : seg + 1]
                            cv = Ct[:, h : h + 1, j : j + 1, seg : seg + 1]
                            o = _owner(c, h, j, seg)
                            if o == "s":
                                nc.scalar.activation(
                                    sl, sl, AF.Identity, bias=cv, scale=a
                                )
                            elif o == "v":
                                nc.vector.tensor_scalar(sl, sl, a, cv, OP.mult, OP.add)
                            else:
                                nc.gpsimd.tensor_scalar(sl, sl, a, cv, OP.mult, OP.add)

            # ---- stores ----
            for h in range(2):
                xg = x_tiles.pop((c, h))
                nc.gpsimd.dma_start(
                    out=out_d[c, 128 * h : 128 * (h + 1), :, :], in_=xg[:, :, :]
                )

    nc.compile()
    return nc


def _get_nc():
    if "nc" not in _NC_CACHE:
        _NC_CACHE["nc"] = build_nc()
    return _NC_CACHE["nc"]


def kernel(x, mask, weights_obj, biases_obj, weights_road, biases_road, _trace=False):
    x = np.asarray(x, dtype=np.float32)
    mask = np.asarray(mask).astype(bool)
    w_obj = np.ascontiguousarray(np.asarray(weights_obj, dtype=np.float32))
    b_obj = np.ascontiguousarray(np.asarray(biases_obj, dtype=np.float32))
    w_road = np.ascontiguousarray(np.asarray(weights_road, dtype=np.float32))
    b_road = np.ascontiguousarray(np.asarray(biases_road, dtype=np.float32))

    xm = np.where(mask[:, :, None], np.float32(0), x).astype(BF)
    # [B, S, D] -> [core, chunk, samp, S, D] -> [core, chunk, D, samp, S]
    xt = np.ascontiguousarray(
        xm.reshape(NCORES, NCHUNK, CHUNK, S, D).transpose(0, 1, 4, 2, 3)
    )
    alive_t = np.ascontiguousarray(
        (~mask).astype(BF).reshape(NCORES, B_LOC, S).transpose(0, 2, 1)
    )

    in_maps = [
        {
            "x": xt[i],
            "alive_t": alive_t[i],
            "weights_obj": w_obj,
            "biases_obj": b_obj,
            "weights_road": w_road,
            "biases_road": b_road,
        }
        for i in range(NCORES)
    ]
    nc = _get_nc()
    res = run_bass_kernel_spmd(nc, in_maps, core_ids=list(range(NCORES)), trace=_trace)
    outs = []
    for i in range(NCORES):
        o = np.asarray(res.results[i]["out"])  # [NCHUNK, D, CHUNK, S] bf16
        outs.append(o.transpose(0, 2, 3, 1).reshape(B_LOC, S, D))
    out = np.concatenate(outs, axis=0).astype(np.float32)
    if _trace:
        kernel.last_exec_time_ns = res.exec_time_ns
        kernel.last_mean_exec_time_ns = res.mean_exec_time_ns
    return out.reshape(B, S, D)


# revision 12
# speedup vs baseline: 1.1594x; 1.0930x over previous
"""CrossSetNorm Trainium2 kernel v6 (8 NeuronCores, batch-parallel).

Problem: x [2048, 328, 256] f32, mask [2048, 328] bool (True = dead).
Two independent masked set-norms over the set dim per sample:
  obj = s in [0, 128), road = s in [128, 328)
out[s,d] = xm[s,d] * A[samp,d] + C[samp,d] with A = istd*w, C = b - mean*A.

v6 design (per core, 256 samples = 8 chunks of 32):
  - HOST: pre-mask xm = x*alive, cast bf16, layout [chunk, D, samp, S]
    (d on partitions, contiguous per-sample rows -> full-rate apply).
  - bn_stats per (sample, seg, h): even/odd 6-tuple recombined in the
    chain (v3 math). 1024 ops on DVE -- the dominant cost; everything
    else is kept off DVE where possible.
  - counts: alive shipped transposed [S, B]; per-sample alive counts via
    3 PE ones-matmuls into PSUM [1,2,256]; count-derived rows computed
    once on DVE and PE-broadcast to 128 partitions per chunk (replaces
    v3's 160 SEL matmuls + pack/transpose machinery).
  - chain h-batched per chunk on [128, 2, 32, 2] f32 views; chunk 0 is
    processed in two 16-sample half-ranges so ScalarE/GpSimd applies
    start ~25us earlier (pipeline warmup).
  - apply: per (sample, seg, h) fused mult-add split across DVE/SC/GP by
    per-chunk tables (DVE-heavy in the last chunks to kill the tail).
  - DMA: 2 loads + 2 stores per chunk of [128, 21KB] (128 descriptors of
    20992B each); SP HWDGE in, gpsimd SWDGE out.
"""
import sys

if "/opt/trn_rl_repo" not in sys.path:
    sys.path.insert(0, "/opt/trn_rl_repo")

from contextlib import ExitStack

import numpy as np
import ml_dtypes

import concourse.bacc as bacc
import concourse.bass as bass
import concourse.tile as tile
from concourse import mybir
from concourse.bass_utils import run_bass_kernel_spmd

F32 = mybir.dt.float32
F32R = mybir.dt.float32r
BF16 = mybir.dt.bfloat16
AF = mybir.ActivationFunctionType
OP = mybir.AluOpType
BF = ml_dtypes.bfloat16

NCORES = 8
B, S, D = 2048, 328, 256
B_LOC = B // NCORES      # 256
S_OBJ = 128
S_ROAD = S - S_OBJ       # 200
CHUNK = 32               # samples per chunk
NCHUNK = B_LOC // CHUNK  # 8
EPS = 1e-6

# apply split per chunk: of each 64 (j, seg) slots per (chunk, h):
# first N_DVE -> DVE, next N_SC -> ScalarE, rest -> GpSimd
N_DVE_TAB = [0, 2, 2, 2, 2, 2, 10, 22]
N_SC_TAB = [40, 39, 39, 39, 39, 39, 36, 30]

_NC_CACHE = {}


def _owner(c, h, j, seg):
    r = (2 * j + seg + 7 * h + 13 * c) % 64
    nd, ns = N_DVE_TAB[c], N_SC_TAB[c]
    if r < nd:
        return "v"
    if r < nd + ns:
        return "s"
    return "g"


def _o(x):
    """Squeeze/merge an AP to minimal dims (sequencer decode cost scales
    with AP dimensionality; deep singleton slices are ~30ns/dim slower)."""
    return x.opt()


def build_nc():
    nc = bacc.Bacc("TRN2", target_bir_lowering=False, debug=False, num_devices=NCORES)
    x_d = nc.declare_dram_parameter("x", [NCHUNK, D, CHUNK, S], BF16, isOutput=False)
    at_d = nc.declare_dram_parameter("alive_t", [S, B_LOC], BF16, isOutput=False)
    w_obj_d = nc.declare_dram_parameter("weights_obj", [D], F32, isOutput=False)
    b_obj_d = nc.declare_dram_parameter("biases_obj", [D], F32, isOutput=False)
    w_road_d = nc.declare_dram_parameter("weights_road", [D], F32, isOutput=False)
    b_road_d = nc.declare_dram_parameter("biases_road", [D], F32, isOutput=False)
    out_d = nc.declare_dram_parameter("out", [NCHUNK, D, CHUNK, S], BF16, isOutput=True)

    with tile.TileContext(nc) as tc, ExitStack() as ctx:
        singles = ctx.enter_context(tc.tile_pool(name="singles", bufs=1))
        xp = ctx.enter_context(tc.tile_pool(name="xp", bufs=6))
        bnp = ctx.enter_context(tc.tile_pool(name="bnp", bufs=2))
        tmp = ctx.enter_context(tc.tile_pool(name="tmp", bufs=2))
        psA = ctx.enter_context(tc.tile_pool(name="psA", bufs=1, space="PSUM"))
        psB = ctx.enter_context(tc.tile_pool(name="psB", bufs=2, space="PSUM"))

        # ---- constants ----
        onesK = singles.tile([128, 1], BF16)
        nc.vector.memset(onesK, 1.0)
        onesM = singles.tile([1, 128], F32)
        nc.vector.memset(onesM, 1.0)
        nhalfT = singles.tile([1, 2, B_LOC], F32)
        nc.vector.memset(nhalfT[:, 0:1, :], float(S_OBJ // 2))
        nc.vector.memset(nhalfT[:, 1:2, :], float(S_ROAD // 2))
        nfullT = singles.tile([1, 2, B_LOC], F32)
        nc.vector.memset(nfullT[:, 0:1, :], float(S_OBJ))
        nc.vector.memset(nfullT[:, 1:2, :], float(S_ROAD))
        W = singles.tile([128, 2, 2], F32)   # [d-half, h, seg]
        Bt = singles.tile([128, 2, 2], F32)
        for h in range(2):
            for seg, (wd, bd) in enumerate(((w_obj_d, b_obj_d), (w_road_d, b_road_d))):
                nc.sync.dma_start(
                    out=W[:, h : h + 1, seg : seg + 1],
                    in_=bass.AP(tensor=wd, offset=128 * h, ap=[[1, 128], [1, 1]]),
                )
                nc.sync.dma_start(
                    out=Bt[:, h : h + 1, seg : seg + 1],
                    in_=bass.AP(tensor=bd, offset=128 * h, ap=[[1, 128], [1, 1]]),
                )
        aT1 = singles.tile([128, 2, B_LOC], BF16)
        nc.sync.dma_start(out=aT1[:, 0:1, :], in_=at_d[0:128, :])
        nc.sync.dma_start(out=aT1[:, 1:2, :], in_=at_d[128:256, :])
        aT2 = singles.tile([128, B_LOC], BF16)
        nc.sync.dma_start(out=aT2[0:72, :], in_=at_d[256:328, :])

        # ---- x loads ----
        x_tiles = {}

        def emit_loads(c):
            for h in range(2):
                t = xp.tile([128, CHUNK, S], BF16, tag="x", name="x")
                nc.sync.dma_start(out=t, in_=x_d[c, 128 * h : 128 * (h + 1), :, :])
                x_tiles[(c, h)] = t

        emit_loads(0)

        # ---- alive counts via PE (contract over s on partitions) ----
        CNT = psA.tile([1, 2, B_LOC], F32, tag="cnt")
        nc.tensor.matmul(CNT[:, 0:1, :], onesK[:, :], aT1[:, 0:1, :], start=True, stop=True)
        nc.tensor.matmul(CNT[:, 1:2, :], onesK[:, :], aT1[:, 1:2, :], start=True, stop=False)
        nc.tensor.matmul(CNT[:, 1:2, :], onesK[0:72, :], aT2[0:72, :], start=False, stop=True)

        # ---- count-derived rows, once per core on [1, 2, 256] ----
        # rows: 0 = nhalf*rc, 1 = nfull*rc - 2, 2 = rc, 3 = okt, 4 = okm
        RF = singles.tile([1, 5, 2, B_LOC], F32)
        cc = singles.tile([1, 2, B_LOC], F32)
        nc.vector.tensor_scalar(cc, CNT, 1.0, None, OP.max)
        nc.vector.reciprocal(RF[:, 2:3, :, :], cc)
        nc.vector.tensor_mul(RF[:, 0:1, :, :], RF[:, 2:3, :, :], nhalfT)
        nc.vector.tensor_mul(RF[:, 1:2, :, :], RF[:, 2:3, :, :], nfullT)
        nc.vector.tensor_scalar(RF[:, 1:2, :, :], RF[:, 1:2, :, :], -2.0, None, OP.add)
        nc.vector.tensor_scalar(RF[:, 3:4, :, :], CNT, -1.0, 1.0, OP.add, OP.min)
        nc.vector.tensor_scalar(RF[:, 3:4, :, :], RF[:, 3:4, :, :], 0.0, None, OP.max)
        nc.vector.tensor_scalar(RF[:, 4:5, :, :], RF[:, 3:4, :, :], -1.0, 1.0, OP.mult, OP.add)
        R = singles.tile([1, 5, 2, NCHUNK, CHUNK], F32R)
        nc.vector.tensor_scalar(R[:, :, :, :, :], RF, 1.0, None, OP.mult)

        for c in range(NCHUNK):
            if c + 1 < NCHUNK:
                emit_loads(c + 1)

            # ---- broadcast count rows to 128 partitions for this chunk ----
            RB = psB.tile([128, 5, 2, CHUNK], F32, tag="rb", name="RB")
            nc.tensor.matmul(
                RB, onesM[:, :].bitcast(F32R), R[0:1, :, :, c, :],
                start=True, stop=True,
            )
            # Rsb: [128, h, 5, samp, seg] (rows duplicated per d-half)
            Rsb = tmp.tile([128, 2, 5, CHUNK, 2], F32, name="rsb")
            for h in range(2):
                for seg in range(2):
                    nc.vector.tensor_scalar(
                        _o(Rsb[:, h : h + 1, :, :, seg : seg + 1]),
                        _o(RB[:, :, seg : seg + 1, :]),
                        1.0, None, OP.mult,
                    )

            BN = bnp.tile([128, 2, CHUNK, 2, 6], F32, tag="bn", name="BN")
            MEAN = tmp.tile([128, 2, CHUNK, 2], F32, name="MEAN")
            A1 = tmp.tile([128, 2, CHUNK, 2], F32, name="A1")
            B1 = tmp.tile([128, 2, CHUNK, 2], F32, name="B1")
            U = tmp.tile([128, 2, CHUNK, 2], F32, name="U")
            ISTD = tmp.tile([128, 2, CHUNK, 2], F32, name="ISTD")
            At = tmp.tile([128, 2, CHUNK, 2], F32, name="At")
            Ct = tmp.tile([128, 2, CHUNK, 2], F32, name="Ct")

            # chunk 0 processed in two half-ranges for faster pipeline start
            ranges = [(0, 16), (16, 32)] if c == 0 else [(0, CHUNK)]
            for (s0, s1) in ranges:
                # ---- bn_stats per (h, sample, seg) ----
                for h in range(2):
                    xg = x_tiles[(c, h)]
                    for j in range(s0, s1):
                        nc.vector.bn_stats(
                            _o(BN[:, h : h + 1, j : j + 1, 0:1, :]),
                            _o(xg[:, j : j + 1, 0:S_OBJ]),
                        )
                        nc.vector.bn_stats(
                            _o(BN[:, h : h + 1, j : j + 1, 1:2, :]),
                            _o(xg[:, j : j + 1, S_OBJ:S]),
                        )

                # ---- stats chain, h-batched on [128, 2, samp, 2] views ----
                def rr(r):
                    return _o(Rsb[:, :, r : r + 1, s0:s1, :])

                def sv(t):
                    return _o(t[:, :, s0:s1, :])
                me = _o(BN[:, :, s0:s1, :, 1:2])
                mo = _o(BN[:, :, s0:s1, :, 4:5])
                Me = _o(BN[:, :, s0:s1, :, 2:3])
                Mo = _o(BN[:, :, s0:s1, :, 5:6])
                nc.vector.tensor_add(sv(U), me, mo)
                nc.vector.tensor_mul(sv(MEAN), sv(U), rr(0))   # mean
                nc.vector.tensor_mul(sv(A1), me, me)
                nc.vector.tensor_mul(sv(B1), mo, mo)
                nc.vector.tensor_add(sv(A1), sv(A1), sv(B1))
                nc.vector.tensor_mul(sv(A1), sv(A1), rr(0))    # (me^2+mo^2)*nh*rc
                nc.vector.tensor_add(sv(B1), Me, Mo)
                nc.vector.tensor_mul(sv(B1), sv(B1), rr(2))    # M2*rc
                nc.vector.tensor_add(sv(A1), sv(A1), sv(B1))   # s2*rc
                nc.vector.tensor_mul(sv(B1), sv(MEAN), sv(MEAN))
                nc.vector.tensor_mul(sv(B1), sv(B1), rr(1))    # mean^2*(n*rc-2)
                nc.vector.tensor_add(sv(A1), sv(A1), sv(B1))   # var
                nc.vector.tensor_scalar(sv(A1), sv(A1), EPS, None, OP.add)
                nc.vector.reciprocal(sv(B1), sv(A1))
                nc.scalar.activation(sv(ISTD), sv(B1), AF.Sqrt)
                nc.vector.tensor_mul(sv(ISTD), sv(ISTD), rr(3))
                nc.vector.tensor_add(sv(ISTD), sv(ISTD), rr(4))
                for h in range(2):
                    for seg in range(2):
                        nc.vector.tensor_scalar(
                            _o(At[:, h : h + 1, s0:s1, seg : seg + 1]),
                            _o(ISTD[:, h : h + 1, s0:s1, seg : seg + 1]),
                            _o(W[:, h : h + 1, seg : seg + 1]), None, OP.mult,
                        )
                nc.vector.tensor_mul(sv(B1), sv(MEAN), sv(At))
                for h in range(2):
                    for seg in range(2):
                        nc.vector.tensor_scalar(
                            _o(Ct[:, h : h + 1, s0:s1, seg : seg + 1]),
                            _o(B1[:, h : h + 1, s0:s1, seg : seg + 1]),
                            -1.0, _o(Bt[:, h : h + 1, seg : seg + 1]), OP.mult, OP.add,
                        )

                # ---- apply ----
                for h in range(2):
                    xg = x_tiles[(c, h)]
                    for j in range(s0, s1):
                        for seg in range(2):
                            if seg == 0:
                                sl = xg[:, j : j + 1, 0:S_OBJ]
                            else:
                                sl = xg[:, j : j + 1, S_OBJ:S]
                            sl = _o(sl)
                            a = _o(At[:, h : h + 1, j : j + 1, seg : seg + 1])
                            cv = _o(Ct[:, h : h + 1, j : j + 1, seg : seg + 1])
                            o = _owner(c, h, j, seg)
                            if o == "s":
                                nc.scalar.activation(
                                    sl, sl, AF.Identity, bias=cv, scale=a
                                )
                            elif o == "v":
                                nc.vector.tensor_scalar(sl, sl, a, cv, OP.mult, OP.add)
                            else:
                                nc.gpsimd.tensor_scalar(sl, sl, a, cv, OP.mult, OP.add)

            # ---- stores ----
            for h in range(2):
                xg = x_tiles.pop((c, h))
                nc.gpsimd.dma_start(
                    out=out_d[c, 128 * h : 128 * (h + 1), :, :], in_=xg[:, :, :]
                )

    nc.compile()
    return nc


def _get_nc():
    if "nc" not in _NC_CACHE:
        _NC_CACHE["nc"] = build_nc()
    return _NC_CACHE["nc"]


def kernel(x, mask, weights_obj, biases_obj, weights_road, biases_road, _trace=False):
    x = np.asarray(x, dtype=np.float32)
    mask = np.asarray(mask).astype(bool)
    w_obj = np.ascontiguousarray(np.asarray(weights_obj, dtype=np.float32))
    b_obj = np.ascontiguousarray(np.asarray(biases_obj, dtype=np.float32))
    w_road = np.ascontiguousarray(np.asarray(weights_road, dtype=np.float32))
    b_road = np.ascontiguousarray(np.asarray(biases_road, dtype=np.float32))

    xm = np.where(mask[:, :, None], np.float32(0), x).astype(BF)
    # [B, S, D] -> [core, chunk, samp, S, D] -> [core, chunk, D, samp, S]
    xt = np.ascontiguousarray(
        xm.reshape(NCORES, NCHUNK, CHUNK, S, D).transpose(0, 1, 4, 2, 3)
    )
    alive_t = np.ascontiguousarray(
        (~mask).astype(BF).reshape(NCORES, B_LOC, S).transpose(0, 2, 1)
    )

    in_maps = [
        {
            "x": xt[i],
            "alive_t": alive_t[i],
            "weights_obj": w_obj,
            "biases_obj": b_obj,
            "weights_road": w_road,
            "biases_road": b_road,
        }
        for i in range(NCORES)
    ]
    nc = _get_nc()
    res = run_bass_kernel_spmd(nc, in_maps, core_ids=list(range(NCORES)), trace=_trace)
    outs = []
    for i in range(NCORES):
        o = np.asarray(res.results[i]["out"])  # [NCHUNK, D, CHUNK, S] bf16
        outs.append(o.transpose(0, 2, 3, 1).reshape(B_LOC, S, D))
    out = np.concatenate(outs, axis=0).astype(np.float32)
    if _trace:
        kernel.last_exec_time_ns = res.exec_time_ns
        kernel.last_mean_exec_time_ns = res.mean_exec_time_ns
    return out.reshape(B, S, D)
